# revision 1
# baseline (speedup 1.0000x reference)
"""LIF (leaky integrate-and-fire) forward kernel for Trainium2, 8 NeuronCores.

Recurrence (per element of [B, N], serial over T):
    v_t = DECAY * (v_{t-1} * (1 - s_{t-1})) + x_t      (REST = 0)
    s_t = (v_t > THRESHOLD)

Reformulated with state w_t = v_t * [v_t <= THRESHOLD] (post-reset membrane):
    v_t = (w_{t-1} * DECAY) + x_t        -> one fused scalar_tensor_tensor (DVE)
    w_t = (v_t is_le THR) * v_t          -> one fused scalar_tensor_tensor (DVE)
    out = Sign(v_t - THR)                -> ScalarE activation, fp8 {-1,0,1}
Host decodes spikes as (out > 0). All arithmetic is fp32 and bitwise-faithful
to the reference ordering.

Sharding: batch dim (128) split 16 rows/core across 8 cores; per-core,
per-step slab is a contiguous 1 MiB block viewed as [128 partitions, 2048].
"""

import numpy as np

import concourse.bacc as bacc
import concourse.mybir as mybir
from concourse.tile import TileContext
from concourse.bass_utils import run_bass_kernel_spmd

T, B, N = 32, 128, 16384
N_CORES = 8
B_SH = B // N_CORES          # 16 batch rows per core
S = B_SH * N                 # 262144 elements per core per time step
P = 128                      # SBUF partitions
F = S // P                   # 2048 free-dim elements
DECAY = 0.2
THR = 0.3

TRACE = False                # set True (e.g. from test.py) to capture a profile

_BUILT = {}


def _build_nc():
    nc = bacc.Bacc("TRN2", debug=False, num_devices=N_CORES)
    x = nc.dram_tensor("x", [T, S], mybir.dt.float32, kind="ExternalInput").ap()
    y = nc.dram_tensor("y", [T, S], mybir.dt.float8e4, kind="ExternalOutput").ap()
    xr = x.rearrange("t (p f) -> t p f", p=P)
    yr = y.rearrange("t (p f) -> t p f", p=P)

    f32 = mybir.dt.float32
    Alu = mybir.AluOpType
    Act = mybir.ActivationFunctionType

    H = F // 2
    with TileContext(nc) as tc:
        with (
            tc.tile_pool(name="state", bufs=1) as state_pool,
            tc.tile_pool(name="xin", bufs=10) as xin_pool,
            tc.tile_pool(name="vtmp", bufs=4) as v_pool,
            tc.tile_pool(name="sout", bufs=8) as s_pool,
        ):
            negthr = nc.alloc_sbuf_tensor("const_negthr", [P, 1], f32).ap()
            nc.gpsimd.memset(negthr, -THR)
            w = state_pool.tile([P, F], f32)
            for t in range(T):
                xt = xin_pool.tile([P, F], f32)
                if t == 0:
                    # split the first load so compute can start sooner
                    nc.sync.dma_start(out=xt[:, :H], in_=xr[t][:, :H])
                    nc.sync.dma_start(out=xt[:, H:], in_=xr[t][:, H:])
                else:
                    nc.sync.dma_start(out=xt[:], in_=xr[t])

                v = v_pool.tile([P, F], f32)
                st = s_pool.tile([P, F], mybir.dt.float8e4)
                if t == 0:
                    # w_{-1}=0 so v_0 = x_0: skip STT-A, read x directly
                    for c0, c1 in ((0, H), (H, F)):
                        nc.vector.scalar_tensor_tensor(
                            out=w[:, c0:c1], in0=xt[:, c0:c1], scalar=THR,
                            in1=xt[:, c0:c1], op0=Alu.is_le, op1=Alu.mult,
                        )
                        nc.scalar.activation(
                            st[:, c0:c1], xt[:, c0:c1], Act.Sign, bias=negthr
                        )
                        nc.sync.dma_start(
                            out=yr[t][:, c0:c1], in_=st[:, c0:c1]
                        )
                elif t == T - 1:
                    # tail latency trim: process in column halves
                    for c0, c1 in ((0, H), (H, F)):
                        nc.vector.scalar_tensor_tensor(
                            out=v[:, c0:c1], in0=w[:, c0:c1], scalar=DECAY,
                            in1=xt[:, c0:c1], op0=Alu.mult, op1=Alu.add,
                        )
                        nc.vector.scalar_tensor_tensor(
                            out=w[:, c0:c1], in0=v[:, c0:c1], scalar=THR,
                            in1=v[:, c0:c1], op0=Alu.is_le, op1=Alu.mult,
                        )
                        nc.scalar.activation(
                            st[:, c0:c1], v[:, c0:c1], Act.Sign, bias=negthr
                        )
                        nc.sync.dma_start(
                            out=yr[t][:, c0:c1], in_=st[:, c0:c1]
                        )
                else:
                    # v = (w * DECAY) + x
                    nc.vector.scalar_tensor_tensor(
                        out=v[:], in0=w[:], scalar=DECAY, in1=xt[:],
                        op0=Alu.mult, op1=Alu.add,
                    )
                    # w = (v is_le THR) * v
                    nc.vector.scalar_tensor_tensor(
                        out=w[:], in0=v[:], scalar=THR, in1=v[:],
                        op0=Alu.is_le, op1=Alu.mult,
                    )
                    # spike encoding: Sign(v-THR) fp8; host decodes (>0)
                    nc.scalar.activation(st[:], v[:], Act.Sign, bias=negthr)
                    nc.sync.dma_start(out=yr[t], in_=st[:])
    nc.compile()
    return nc


LAST_RESULTS = None


def kernel(tx):
    global LAST_RESULTS
    tx = np.asarray(tx)
    assert tx.shape == (T, B, N) and tx.dtype == np.float32

    if "nc" not in _BUILT:
        _BUILT["nc"] = _build_nc()
    nc = _BUILT["nc"]

    in_maps = [
        {"x": np.ascontiguousarray(tx[:, c * B_SH:(c + 1) * B_SH, :]).reshape(T, S)}
        for c in range(N_CORES)
    ]
    res = run_bass_kernel_spmd(nc, in_maps, core_ids=list(range(N_CORES)), trace=TRACE)
    LAST_RESULTS = res

    out = np.empty((T, B, N), dtype=np.float32)
    for c in range(N_CORES):
        sgn = np.asarray(res.results[c]["y"]).reshape(T, B_SH, N)
        out[:, c * B_SH:(c + 1) * B_SH, :] = (sgn > 0).astype(np.float32)
    return out

